# revision 5
# baseline (speedup 1.0000x reference)
"""Trainium2 Bass kernel for nn_Attn_25417616458107 (sparse_attention).

Reference computation:
    energy[s,b,:] = enc[s,b,:] @ W^T + b_attn          # [S,B,H]
    score[b,s]    = hidden[0,b,:] . energy[s,b,:]       # [B,S]
    out           = softmax(score, axis=s)[:, None, :]  # [B,1,S]

Key algebraic reformulation: reassociating the two contractions,
    score[b,s] = (hidden[0,b,:] @ W) . enc[s,b,:] + hidden[0,b,:].b_attn
The bias term is constant per row b, so it cancels in the softmax.  With
q = hidden[0] @ W (a tiny [B,H]x[H,H] matmul done on the host), the device
kernel reduces to a batched dot-product stream over encoder_outputs plus a
row softmax -- memory-bound instead of the naive 275-GFLOP einsum.

Sharding: data-parallel over batch.  Each of the 8 cores gets 8 of the 64
batches: enc shard [S=2048, 8, H=1024] plus its q rows (pre-replicated to
the [128, 8, 1024] SBUF operand layout).  No cross-core communication.

On-chip layout per core:
    tile t (16 total) covers s in [128t, 128t+128), s = 128t + 8*sa + sb
    SBUF tile [partition=(b*16+sa), free=(sb,h)]  (4 MiB, 4KiB bursts)
    DVE: one tensor_tensor mult with q2 (in place)       16 x 8.7us
    ACT: per sb, Copy-activation with accum_out -> score  128 x 1.0us
    scores land in an SBUF [128, 128] tile; a DRAM bounce re-lays them as
    rows [b, s] for the softmax (max/exp+sum/scale) and the output DMA.
"""

import sys
import numpy as np

_S, _B, _H = 2048, 64, 1024
_NCORES = 8
_BLOC = _B // _NCORES  # 8 batches per core
_SA, _SB = 16, 8       # s = 128*t + 8*sa + sb; partition=(b,sa), free=(sb,h)
_NT = _S // (_SA * _SB)  # 16 tiles

_cache = {}


def _concourse():
    if "/opt/trn_rl_repo" not in sys.path:
        sys.path.insert(0, "/opt/trn_rl_repo")


def _build():
    _concourse()
    import concourse.bacc as bacc
    import concourse.mybir as mybir
    import concourse.tile as tile

    f32 = mybir.dt.float32
    nc = bacc.Bacc("TRN2", target_bir_lowering=False, debug=False)

    enc = nc.dram_tensor("enc", [_S, _BLOC, _H], f32, kind="ExternalInput")
    q2 = nc.dram_tensor("q2", [128, _SB, _H], f32, kind="ExternalInput")
    out = nc.dram_tensor("out", [_BLOC, _S], f32, kind="ExternalOutput")
    scratch = nc.dram_tensor("scratch", [128, _NT * _SB], f32)

    # tile t: partition p = b*16+sa, free f = sb*H+h  <->  enc[t*128 + sa*8 + sb, b, h]
    # (sa, sb) are stride-adjacent in s, so the DRAM side merges to 3 AP dims.
    enc_r = enc.rearrange("(t sa sb) b h -> t b sa sb h", sa=_SA, sb=_SB)
    # scratch[b*16+sa, t*8+sb] -> rows[b, s] with s = t*128 + sa*8 + sb,
    # read back per-t (16 small DMAs) to stay within the 3-dim DMA AP limit.
    sc_rows = scratch.rearrange("(b sa) (t sb) -> t b sa sb", sa=_SA, t=_NT)

    with tile.TileContext(nc) as tc:
        with (
            tc.tile_pool(name="encp", bufs=3) as encp,
            tc.tile_pool(name="qp", bufs=1) as qp,
            tc.tile_pool(name="dumpp", bufs=2) as dumpp,
            tc.tile_pool(name="smallp", bufs=1) as smallp,
        ):
            q2t = qp.tile([128, _SB, _H], f32)
            nc.sync.dma_start(q2t[:], q2[:])

            scores = smallp.tile([128, _NT * _SB], f32)

            for t in range(_NT):
                et = encp.tile([128, _SB, _H], f32, tag="enc")
                nc.sync.dma_start(et[:], enc_r[t])
                nc.vector.tensor_mul(et[:], et[:], q2t[:])
                for sb in range(_SB):
                    dump = dumpp.tile([128, 1, _H], f32, tag="dump")
                    nc.scalar.activation(
                        dump[:],
                        et[:, sb : sb + 1, :],
                        mybir.ActivationFunctionType.Copy,
                        accum_out=scores[:, t * _SB + sb : t * _SB + sb + 1],
                    )

            nc.sync.dma_start(scratch[:], scores[:])
            rows = smallp.tile([_BLOC, _S], f32)
            rows_t = rows.rearrange("b (t sa sb) -> t b sa sb", t=_NT, sa=_SA)
            for t in range(_NT):
                nc.sync.dma_start(rows_t[t], sc_rows[t])

            negmx = smallp.tile([_BLOC, 1], f32)
            nc.vector.tensor_reduce(
                negmx[:],
                rows[:],
                axis=mybir.AxisListType.X,
                op=mybir.AluOpType.max,
                negate=True,
            )
            erows = smallp.tile([_BLOC, _S], f32)
            zsum = smallp.tile([_BLOC, 1], f32)
            nc.scalar.activation(
                erows[:],
                rows[:],
                mybir.ActivationFunctionType.Exp,
                bias=negmx[:],
                scale=1.0,
                accum_out=zsum[:],
            )
            rz = smallp.tile([_BLOC, 1], f32)
            nc.vector.reciprocal(rz[:], zsum[:])
            nc.vector.tensor_scalar_mul(erows[:], erows[:], rz[:])
            nc.sync.dma_start(out[:], erows[:])

    nc.compile()
    return nc


def _in_maps(hidden, encoder_outputs, W_attn):
    hidden = np.asarray(hidden, dtype=np.float32)
    enc = np.asarray(encoder_outputs, dtype=np.float32)
    W = np.asarray(W_attn, dtype=np.float32)
    q = hidden[0] @ W  # [B, H]; bias term is constant per row -> cancels in softmax
    maps = []
    for c in range(_NCORES):
        bsl = slice(c * _BLOC, (c + 1) * _BLOC)
        q2 = np.repeat(q[bsl], _SA, axis=0)  # [128, H], partition (b, sa)
        q2e = np.ascontiguousarray(
            np.broadcast_to(q2[:, None, :], (128, _SB, _H)), dtype=np.float32
        )
        maps.append(
            {
                "enc": np.ascontiguousarray(enc[:, bsl, :]),
                "q2": q2e,
            }
        )
    return maps


def kernel(hidden, encoder_outputs, W_attn, b_attn, **_unused):
    _concourse()
    from concourse.bass_utils import run_bass_kernel_spmd

    if "nc" not in _cache:
        _cache["nc"] = _build()
    nc = _cache["nc"]

    maps = _in_maps(hidden, encoder_outputs, W_attn)
    res = run_bass_kernel_spmd(nc, maps, core_ids=list(range(_NCORES)))
    outs = [np.asarray(res.results[c]["out"]) for c in range(_NCORES)]
    full = np.concatenate(outs, axis=0)  # [B, S]
    return full[:, None, :].astype(np.float32)
